# revision 25
# baseline (speedup 1.0000x reference)
"""MoE layer (top-2 routing, 8 experts) for Trainium2 across 8 NeuronCores.

Strategy (expert-parallel):
  - The gate (logits = x @ Wg, top-2 + softmax) is computed on host; it is
    ~0.03% of the layer's FLOPs. Tokens are gathered per selected expert on
    host (the "all-to-all dispatch" of the sharding hint, done at input
    sharding time), padded to a common capacity C, one expert per core.
  - Each core runs its expert's FFN on its routed tokens:
        yT = (gelu(W1.T-tiled matmuls of x) @ W2 + b2) * gate_weight
    entirely in transposed [feature, token] layout so no on-chip transposes
    are needed. Matmul inputs are bf16 (f32 PSUM accumulation).
  - Host combines: out[t] = contrib(expert0(t)) + contrib(expert1(t)) via two
    vectorized gathers (gate weights were already applied on-device).

Per-core compute: 2 * C * D * H MACs (C = max expert count, ~1091 for the
reference routing) -> ~18.3 GFLOP -> ~295 us at the measured 2.0 GHz PE
clock; measured ~298 us/iteration on HW (~99% of that roofline).
"""

import os
from contextlib import ExitStack

import ml_dtypes
import numpy as np

import concourse.bass as bass
import concourse.tile as tile
from concourse import bacc, mybir
from concourse.bass_utils import run_bass_kernel_spmd

# Under axon without the NTFF hook module, trace=True would crash on import.
try:  # pragma: no cover
    import antenv.axon_hooks  # noqa: F401
except ImportError:
    os.environ.setdefault("BASS_NEVER_TRACE", "1")

BF16 = ml_dtypes.bfloat16
D, H, O, E, TOPK = 1024, 4096, 1024, 8, 2
P = 128
N_CORES = 8
N_D, N_H, N_O = D // P, H // P, O // P  # 8, 32, 8 k/m tiles

_CACHE: dict[int, bass.Bass] = {}


def _token_tiles(C):
    """Split capacity C into near-equal moving-dim chunks <= 512.

    Equal chunks beat a [512, ..., small-tail] split: a matmul's issue rate is
    max(N/2.4GHz, LDWEIGHTS ~107ns), so chunks below ~256 are LDW-bound and
    waste PE cycles. E.g. C=1152: 3x384 = ~246us vs [512,512,128] = ~273us.
    """
    n_chunks = -(-C // 512)
    base, rem = divmod(C, n_chunks)
    tiles, t0 = [], 0
    for i in range(n_chunks):
        n = base + (1 if i < rem else 0)
        tiles.append((t0, n))
        t0 += n
    return tiles


def _dedup_ldweights(nc, enabled: bool = True) -> int:
    """Remove InstLdweights whose weights-AP matches the previous PE weight
    load and which carry no sync info — the PE weight buffer already holds
    that tile, so the reload is a pure waste of PE issue slots.

    PE executes its instructions in block order; LDW/MM are the only PE
    instructions here and matmuls never modify the weight buffer.
    """
    if not enabled:
        return 0
    removed = 0
    for blk in nc.m.functions[0].blocks:
        last_key = None
        keep = []
        for inst in blk.instructions:
            if isinstance(inst, mybir.InstLdweights):
                si = inst.sync_info
                clean = si is None or (not si.on_wait and not si.on_update)
                key = (str(inst.ins[0]), str(inst.perf_mode),
                       str(inst.tile_position))
                if clean and key == last_key:
                    removed += 1
                    continue
                last_key = key
            elif isinstance(inst, mybir.InstMatmult):
                pass
            elif not isinstance(inst, (mybir.InstDMACopy, mybir.InstActivation,
                                       mybir.InstTensorTensor)):
                # control flow / drains / barriers: be conservative
                last_key = None
            keep.append(inst)
        blk.instructions[:] = keep
    return removed


def _build(C: int, iters: int = 1, degenerate_w: bool = False) -> bass.Bass:
    """One expert's FFN over C routed tokens, [feature, token] layout.

    Inputs (per core): xt [D, C] bf16 (tokens transposed), w1 [D, H] bf16,
    w2 [H, O] bf16, b1 [H] f32, b2 [O] f32, g [C] f32 (per-token gate weight,
    zero for padding). Output: yt [O, C] f32 = ((gelu(x@w1+b1))@w2 + b2) * g.
    """
    f32, bf16 = mybir.dt.float32, mybir.dt.bfloat16
    nc = bacc.Bacc("TRN2", target_bir_lowering=False, debug=False,
                   num_devices=N_CORES)
    xt_d = nc.dram_tensor("xt", [D, C], bf16, kind="ExternalInput").ap()
    w1_d = nc.dram_tensor("w1", [D, H], bf16, kind="ExternalInput").ap()
    w2_d = nc.dram_tensor("w2", [H, O], bf16, kind="ExternalInput").ap()
    # b1/b2 arrive pre-transposed as [128, m] with b[p, m] = bias[m*128 + p];
    # g arrives pre-broadcast as [128, C] (plain contiguous DMAs — fancy
    # strided/broadcast DMA patterns fan out across queues and blow the
    # per-instruction sync-wait limit on their first consumer).
    b1_d = nc.dram_tensor("b1", [P, N_H], f32, kind="ExternalInput").ap()
    b2_d = nc.dram_tensor("b2", [P, N_O], f32, kind="ExternalInput").ap()
    g_d = nc.dram_tensor("g", [P, C], f32, kind="ExternalInput").ap()
    yt_d = nc.dram_tensor("yt", [O, C], f32, kind="ExternalOutput").ap()

    with tile.TileContext(nc) as tc, ExitStack() as ctx:
        wpool = ctx.enter_context(tc.tile_pool(name="weights", bufs=1))
        xpool = ctx.enter_context(tc.tile_pool(name="xin", bufs=1))
        hpool = ctx.enter_context(tc.tile_pool(name="hts", bufs=34))
        ppool1 = ctx.enter_context(tc.tile_pool(name="ps1", bufs=2, space="PSUM"))
        ppool2 = ctx.enter_context(tc.tile_pool(name="ps2", bufs=2, space="PSUM"))
        ypool = ctx.enter_context(tc.tile_pool(name="yout", bufs=2))

        w1_sb = wpool.tile([P, N_D, H], bf16)   # 64 KB/partition
        w2_sb = wpool.tile([P, N_H, O], bf16)   # 64 KB/partition
        b1_sb = wpool.tile([P, N_H], f32)
        b2_sb = wpool.tile([P, N_O], f32)
        g_sb = wpool.tile([P, C], f32)

        # DMA emission order == HWDGE queue order == consumption order:
        # tile-0 activations and biases first (the first matmuls need them),
        # then W1 chunked as phase A walks it, then W2 (first needed ~80us in).
        token_tiles = _token_tiles(C)
        xt_tiles = {}
        if iters == 1:
            (t0_first, nt_first) = token_tiles[0]
            xt_sb = xpool.tile([P, N_D, 512], bf16, tag="xt", name="xt0")
            for d in range(N_D):
                nc.sync.dma_start(
                    out=xt_sb[:, d, :nt_first],
                    in_=xt_d[d * P:(d + 1) * P, t0_first:t0_first + nt_first])
            xt_tiles[0] = xt_sb
        for hc in range(8):
            c0, c1 = hc * 512, (hc + 1) * 512
            for d in range(N_D):
                nc.sync.dma_start(out=w1_sb[:, d, c0:c1],
                                  in_=w1_d[d * P:(d + 1) * P, c0:c1])
            if hc == 0:
                # b1 is first read at the first gelu (~13us in); b2/g later.
                nc.sync.dma_start(out=b1_sb[:], in_=b1_d[:])
                nc.sync.dma_start(out=b2_sb[:], in_=b2_d[:])
        for h in range(N_H):
            nc.sync.dma_start(out=w2_sb[:, h, :],
                              in_=w2_d[h * P:(h + 1) * P, :])
        nc.sync.dma_start(out=g_sb[:], in_=g_d[:])

        gelu = mybir.ActivationFunctionType.Gelu
        copy = mybir.ActivationFunctionType.Identity

        loop_ctx = ExitStack()
        if iters > 1:
            # timing-only variant: repeat the whole compute body on-device so
            # (wall(iters) - wall(1)) / (iters - 1) isolates HW exec time from
            # the axon dispatch/data-shipping overhead.
            loop_ctx.enter_context(tc.For_i(0, iters, 1))
        ctx.enter_context(loop_ctx)

        for it, (t0, nt) in enumerate(token_tiles):
            if it in xt_tiles:
                xt_sb = xt_tiles[it]
            else:
                xt_sb = xpool.tile([P, N_D, 512], bf16, tag="xt",
                                   name=f"xt{it}")
                for d in range(N_D):
                    nc.sync.dma_start(out=xt_sb[:, d, :nt],
                                      in_=xt_d[d * P:(d + 1) * P, t0:t0 + nt])
            # Phase A: hT[m*128:(m+1)*128, t] = gelu(x @ w1 + b1) per h-tile
            hts = []
            for m in range(N_H):
                ps = ppool1.tile([P, 512], f32, tag="ps1")
                for d in range(N_D):
                    lw = (w1_sb[:, 0, 0:P] if degenerate_w
                          else w1_sb[:, d, m * P:(m + 1) * P])
                    nc.tensor.matmul(ps[:, :nt], lhsT=lw,
                                     rhs=xt_sb[:, d, :nt],
                                     start=(d == 0), stop=(d == N_D - 1))
                ht = hpool.tile([P, 512], bf16, tag="ht")
                nc.scalar.activation(ht[:, :nt], ps[:, :nt], gelu,
                                     bias=b1_sb[:, m:m + 1])
                hts.append(ht)
            # Phase B: yT[o*128:(o+1)*128, t] = (hT.T-contraction @ w2 + b2)*g
            for o in range(N_O):
                ps2 = ppool2.tile([P, 512], f32, tag="ps2")
                for h in range(N_H):
                    lw = (w1_sb[:, 0, 0:P] if degenerate_w
                          else w2_sb[:, h, o * P:(o + 1) * P])
                    nc.tensor.matmul(ps2[:, :nt], lhsT=lw,
                                     rhs=hts[h][:, :nt],
                                     start=(h == 0), stop=(h == N_H - 1))
                yb = ypool.tile([P, 512], f32, tag="yb")
                nc.scalar.activation(yb[:, :nt], ps2[:, :nt], copy,
                                     bias=b2_sb[:, o:o + 1])
                ym = ypool.tile([P, 512], f32, tag="ym")
                nc.vector.tensor_mul(ym[:, :nt], yb[:, :nt],
                                     g_sb[:, t0:t0 + nt])
                nc.sync.dma_start(out=yt_d[o * P:(o + 1) * P, t0:t0 + nt],
                                  in_=ym[:, :nt])
    nc.compile()
    return nc


def _prepare(x, Wg, W1, b1, W2, b2):
    """Host-side gating + per-expert gather. Returns (in_maps, glob, C, B, S)."""
    B, S, Dx = x.shape
    assert Dx == D and Wg.shape == (D, E), (x.shape, Wg.shape)
    T = B * S
    xf = np.ascontiguousarray(x.reshape(T, D), dtype=np.float32)
    logits = xf.astype(np.float64) @ Wg.astype(np.float64)
    top_i = np.argpartition(-logits, TOPK - 1, axis=1)[:, :TOPK]  # [T, 2]
    lv = np.take_along_axis(logits, top_i, axis=1)
    lv -= lv.max(axis=1, keepdims=True)
    ex = np.exp(lv)
    w = ex / ex.sum(axis=1, keepdims=True)  # [T, 2] softmax over the pair

    flat_e = top_i.reshape(-1)      # pair p = 2*t + k -> expert id
    flat_w = w.reshape(-1)
    counts = np.bincount(flat_e, minlength=E)
    # Tokens are the matmul free dim, so capacity needs no alignment at all;
    # every extra padded token costs PE time on all 8 cores.
    C = max(1024, int(counts.max()))

    xt_bf = np.ascontiguousarray(xf.T).astype(BF16)  # [D, T]
    W1b = W1.astype(BF16)
    W2b = W2.astype(BF16)

    in_maps = []
    glob = np.empty(2 * T, dtype=np.int64)  # pair -> row in stacked outputs
    for e in range(E):
        sel = np.nonzero(flat_e == e)[0]
        tok = sel >> 1
        n = len(sel)
        xt_e = np.zeros((D, C), dtype=BF16)
        xt_e[:, :n] = xt_bf[:, tok]
        g_e = np.zeros((C,), dtype=np.float32)
        g_e[:n] = flat_w[sel]
        glob[sel] = e * C + np.arange(n)
        in_maps.append({
            "xt": xt_e,
            "w1": np.ascontiguousarray(W1b[e]),
            "w2": np.ascontiguousarray(W2b[e]),
            # [128, m] with b[p, m] = bias[m*128 + p]
            "b1": np.ascontiguousarray(
                np.asarray(b1[e], dtype=np.float32).reshape(N_H, P).T),
            "b2": np.ascontiguousarray(
                np.asarray(b2[e], dtype=np.float32).reshape(N_O, P).T),
            # gate weights broadcast across the 128 partitions
            "g": np.ascontiguousarray(np.broadcast_to(g_e, (P, C))),
        })
    return in_maps, glob, C, B, S


def _get_nc(C: int, iters: int = 1) -> bass.Bass:
    key = (C, iters)
    nc = _CACHE.get(key)
    if nc is None:
        nc = _CACHE[key] = _build(C, iters)
    return nc


def _combine(results, glob, C, B, S):
    Y = np.stack([np.asarray(r["yt"]).T for r in results])  # [E, C, O]
    Yflat = Y.reshape(E * C, O)
    out = Yflat[glob[0::2]] + Yflat[glob[1::2]]
    return out.reshape(B, S, O).astype(np.float32, copy=False)


def kernel(x, Wg, W1, b1, W2, b2):
    in_maps, glob, C, B, S = _prepare(x, Wg, W1, b1, W2, b2)
    nc = _get_nc(C)
    res = run_bass_kernel_spmd(nc, in_maps, core_ids=list(range(N_CORES)))
    return _combine(res.results, glob, C, B, S)
